# revision 2
# baseline (speedup 1.0000x reference)
"""Fused GEMM + bias + logsumexp + 2x leaky_relu + 2x exact-gelu kernel for TRN2.

Problem: x:(32768,2048)f16, W:(2048,2048)f16, bias:(2048,)f16
  y = x @ W + bias            (M, N)
  z = logsumexp(y, axis=1)    (M, 1)
  z = leaky_relu(leaky_relu(z, 0.01), 0.01)
  z = gelu(gelu(z, exact))    -> (M, 1) f16

Sharding: data-parallel over M across 8 cores (4096 rows each); W and bias
replicated. No cross-core communication; logsumexp reduces over N locally.

Per-core kernel: W lives in SBUF whole (64KB/partition). x arrives via
DMA-transpose in 512-row super-blocks as xT tiles [128k x 512m]. Per 128-row
m-tile: 64 matmuls ([128,128]x[128,512], 16 k-steps x 4 psum banks), then a
DVE bias-add (psum->sbuf f16), row-max, one ACT exp pass with accumulated
row-sum, and a tiny [128,1] tail (ln, +max, lrelu^2, erf-based gelu^2).
"""

import numpy as np

import concourse.bass as bass
import concourse.tile as tile
from concourse import bacc, mybir
from concourse.bass_utils import run_bass_kernel_spmd

M, K, N = 32768, 2048, 2048
N_CORES = 8
M_SHARD = M // N_CORES  # 4096
P = 128
FREE = 512              # matmul moving free dim = one PSUM bank of f32
KT = K // P             # 16 k-subtiles
NB = N // FREE          # 4 psum banks per m-tile

f16 = mybir.dt.float16
f32 = mybir.dt.float32
AF = mybir.ActivationFunctionType
ALU = mybir.AluOpType

SQRT1_2 = 0.7071067811865476
ERF_CLIP = 5.9  # erf(5.9) == 1.0 to fp32 precision; clamp keeps ACT table in range


def build_program(m_shard=M_SHARD, num_devices=N_CORES):
    nc = bacc.Bacc(
        "TRN2",
        target_bir_lowering=False,
        debug=False,
        enable_asserts=False,
        num_devices=num_devices,
    )
    x = nc.dram_tensor("x", [m_shard, K], f16, kind="ExternalInput").ap()
    W = nc.dram_tensor("W", [K, N], f16, kind="ExternalInput").ap()
    bias = nc.dram_tensor("bias", [N], f16, kind="ExternalInput").ap()
    out = nc.dram_tensor("out", [m_shard, 1], f16, kind="ExternalOutput").ap()

    SBL = 512 if m_shard % 512 == 0 else P  # super-block rows per xT load
    MI = SBL // P                           # m-tiles per super-block
    NSB = m_shard // SBL                    # super-blocks
    MT = m_shard // P                       # total m-tiles

    with tile.TileContext(nc) as tc:
        with (
            tc.tile_pool(name="wpool", bufs=1) as wpool,
            tc.tile_pool(name="xpool", bufs=2) as xpool,
            tc.tile_pool(name="epool", bufs=3) as epool,
            tc.tile_pool(name="spool", bufs=8) as spool,
            tc.tile_pool(name="opool", bufs=1) as opool,
            tc.tile_pool(name="pspool", bufs=8, space="PSUM") as pspool,
        ):
            W_sb = wpool.tile([P, KT, N], f16)
            nc.sync.dma_start(W_sb[:], W.rearrange("(ko p) n -> p ko n", p=P))
            bias_sb = wpool.tile([P, N], f16)
            nc.sync.dma_start(bias_sb[:], bias[None, :].to_broadcast((P, N)))
            out_sb = opool.tile([P, MT], f16)

            for sb in range(NSB):
                xT = xpool.tile([P, KT, SBL], f16)
                for k in range(KT):
                    nc.sync.dma_start_transpose(
                        xT[:, k, :], x[bass.ds(sb * SBL, SBL), bass.ts(k, P)]
                    )
                for mi in range(MI):
                    t = sb * MI + mi
                    pss = [
                        pspool.tile([P, FREE], f32, tag="ps", name=f"ps{t}_{nb}")
                        for nb in range(NB)
                    ]
                    for k in range(KT):
                        lhsT = xT[:, k, bass.ts(mi, P)]
                        for nb in range(NB):
                            nc.tensor.matmul(
                                pss[nb][:],
                                lhsT,
                                W_sb[:, k, bass.ts(nb, FREE)],
                                start=(k == 0),
                                stop=(k == KT - 1),
                            )
                    # y = psum + bias (f16, like the reference's fp16 GEMM output)
                    y = epool.tile([P, N], f16, tag="y")
                    for nb in range(NB):
                        nc.vector.tensor_tensor(
                            y[:, bass.ts(nb, FREE)],
                            pss[nb][:],
                            bias_sb[:, bass.ts(nb, FREE)],
                            ALU.add,
                        )
                    negmax = spool.tile([P, 1], f32, tag="negmax")
                    nc.vector.reduce_max(
                        negmax[:], y[:, :], axis=mybir.AxisListType.X, negate=True
                    )
                    # exp(y - max), row-sum via ACT accumulator
                    ejunk = epool.tile([P, N], f16, tag="ejunk")
                    sumexp = spool.tile([P, 1], f32, tag="sumexp")
                    nc.scalar.activation(
                        ejunk[:], y[:, :], AF.Exp, bias=negmax[:], accum_out=sumexp[:]
                    )
                    z = spool.tile([P, 1], f32, tag="z")
                    nc.scalar.activation(z[:], sumexp[:], AF.Ln)
                    # z = ln(sumexp) + max
                    nc.vector.tensor_tensor(z[:], z[:], negmax[:], ALU.subtract)
                    # leaky_relu(z, 0.01) = max(z, 0.01*z), twice
                    w1 = spool.tile([P, 1], f32, tag="w1")
                    for _ in range(2):
                        nc.vector.tensor_scalar_mul(w1[:], z[:], 0.01)
                        nc.vector.tensor_tensor(z[:], z[:], w1[:], ALU.max)
                    # gelu(z) = 0.5*z*(1+erf(z/sqrt(2))), twice
                    for _ in range(2):
                        u = spool.tile([P, 1], f32, tag="u")
                        nc.vector.tensor_scalar(
                            u[:], z[:], SQRT1_2, ERF_CLIP, ALU.mult, ALU.min
                        )
                        nc.vector.tensor_scalar_max(u[:], u[:], -ERF_CLIP)
                        e = spool.tile([P, 1], f32, tag="e")
                        nc.scalar.activation(e[:], u[:], AF.Erf)
                        nc.vector.tensor_tensor(e[:], z[:], e[:], ALU.mult)
                        nc.vector.tensor_tensor(z[:], z[:], e[:], ALU.add)
                        nc.vector.tensor_scalar_mul(z[:], z[:], 0.5)
                    nc.vector.tensor_copy(out_sb[:, t : t + 1], z[:])

            nc.sync.dma_start(out.rearrange("(t p) o -> p (t o)", p=P), out_sb[:])

    nc.compile()
    return nc


_prog_cache = {}
LAST_RESULTS = None


def kernel(x, W, bias):
    global LAST_RESULTS
    x = np.ascontiguousarray(x)
    W = np.ascontiguousarray(W)
    bias = np.ascontiguousarray(bias)
    assert x.shape == (M, K) and W.shape == (K, N) and bias.shape == (N,)

    key = (M_SHARD, N_CORES)
    if key not in _prog_cache:
        _prog_cache[key] = build_program(*key)
    nc = _prog_cache[key]

    shards = np.split(x, N_CORES, axis=0)
    in_maps = [{"x": s, "W": W, "bias": bias} for s in shards]
    res = run_bass_kernel_spmd(nc, in_maps, list(range(N_CORES)))
    LAST_RESULTS = res
    return np.concatenate([res.results[i]["out"] for i in range(N_CORES)], axis=0)
